# revision 2
# baseline (speedup 1.0000x reference)
"""Cost-volume kernel for Trainium2 (8 NeuronCores, SPMD).

cost[b,c,h,x,d] = left[b,c,h,x] - right[b,c,h,x-d]  (0 where x < d)
with B,C,H,W = 4,32,128,240 and D = 24.

Sharding: every (b,c,h) row is independent, so flatten to 16384 rows of
W=240 and give each of the 8 cores a contiguous 2048-row block (pure
data parallelism, no halo).

Per 128-row SBUF tile the [W, D] cost slab is produced by 6 vector ops.
The output free-axis layout j = 24*w + d is fixed by HBM, and fp32
writes with stride 96 B run ~6x slower on the DVE than contiguous ones,
while strides <= 32 B run at full speed.  So d is split into 3 groups
of 8 (32 B-aligned, 8-contiguous runs):

  rect op (per group g):  w in [8g+7, 240), dg in [0,8):
      ot[24w + 8g + dg] = lt[w] - rt[w - 8g - dg]     (all valid)
  corner op (per group):  the remaining 56 valid cells w in [8g, 8g+7)
      via a sheared AP (dg, k'=w-8g-dg), strided but tiny.

Invalid positions (x < d, all with j < 552) are never written by either
op, so they are zeroed once per buffer at kernel start and persist.
The store DMA moves a fully contiguous [128, 5760] slab per tile.
"""

import sys

if "/opt/trn_rl_repo" not in sys.path:
    sys.path.insert(0, "/opt/trn_rl_repo")

import numpy as np

B, C, H, W, D = 4, 32, 128, 240, 24
P = 128
N_CORES = 8
ROWS = B * C * H                 # 16384
ROWS_PER_CORE = ROWS // N_CORES  # 2048
OTW = W * D                      # 5760

_nc_cache = None


def _build():
    from concourse import mybir, bacc
    import concourse.tile as tile
    import bass_rust

    f32 = mybir.dt.float32
    nc = bacc.Bacc("TRN2", target_bir_lowering=False, debug=False)
    left = nc.dram_tensor("left", [ROWS_PER_CORE, W], f32, kind="ExternalInput").ap()
    right = nc.dram_tensor("right", [ROWS_PER_CORE, W], f32, kind="ExternalInput").ap()
    out = nc.dram_tensor("out", [ROWS_PER_CORE, OTW], f32, kind="ExternalOutput").ap()
    ntiles = ROWS_PER_CORE // P  # 16
    NB = 4
    with tile.TileContext(nc) as tc:
        with tc.tile_pool(name="p", bufs=1) as pool:
            lts = [pool.tile([P, W], f32, name=f"lt{i}") for i in range(NB)]
            rts = [pool.tile([P, W], f32, name=f"rt{i}") for i in range(NB)]
            ots = [pool.tile([P, OTW], f32, name=f"ot{i}") for i in range(NB)]
            for i in range(NB):
                # invalid (x < d) positions all lie in [0, 552); zeroed once,
                # never overwritten by the valid-only compute ops below
                nc.vector.memset(ots[i][:, :552], 0.0)
            for t in range(ntiles):
                lt, rt, ot = lts[t % NB], rts[t % NB], ots[t % NB]
                nc.scalar.dma_start(out=lt[:], in_=left[t * P:(t + 1) * P, :])
                nc.scalar.dma_start(out=rt[:], in_=right[t * P:(t + 1) * P, :])
                for g in range(3):
                    cw = W - (8 * g + 7)
                    o_ap = bass_rust.AP(tensor=ot[:].tensor, offset=200 * g + 168,
                                        ap=[[OTW, P], [24, cw], [1, 8]])
                    l_ap = bass_rust.AP(tensor=lt[:].tensor, offset=8 * g + 7,
                                        ap=[[W, P], [1, cw], [0, 8]])
                    r_ap = bass_rust.AP(tensor=rt[:].tensor, offset=7,
                                        ap=[[W, P], [1, cw], [-1, 8]])
                    nc.vector.tensor_sub(out=o_ap, in0=l_ap, in1=r_ap)
                    o2 = bass_rust.AP(tensor=ot[:].tensor, offset=200 * g,
                                      ap=[[OTW, P], [25, 8], [24, 7]])
                    l2 = bass_rust.AP(tensor=lt[:].tensor, offset=8 * g,
                                      ap=[[W, P], [1, 8], [1, 7]])
                    r2 = bass_rust.AP(tensor=rt[:].tensor, offset=0,
                                      ap=[[W, P], [0, 8], [1, 7]])
                    nc.vector.tensor_sub(out=o2, in0=l2, in1=r2)
                nc.sync.dma_start(out=out[t * P:(t + 1) * P, :], in_=ot[:])
    nc.compile()
    return nc


def _get_nc():
    global _nc_cache
    if _nc_cache is None:
        _nc_cache = _build()
    return _nc_cache


def kernel(left_img: np.ndarray, right_img: np.ndarray) -> np.ndarray:
    from concourse.bass_utils import run_bass_kernel_spmd

    nc = _get_nc()
    lf = np.ascontiguousarray(left_img, dtype=np.float32).reshape(ROWS, W)
    rf = np.ascontiguousarray(right_img, dtype=np.float32).reshape(ROWS, W)
    in_maps = []
    for i in range(N_CORES):
        sl = slice(i * ROWS_PER_CORE, (i + 1) * ROWS_PER_CORE)
        in_maps.append({"left": np.ascontiguousarray(lf[sl]),
                        "right": np.ascontiguousarray(rf[sl])})
    res = run_bass_kernel_spmd(nc, in_maps, list(range(N_CORES)))
    shards = [res.results[i]["out"] for i in range(N_CORES)]
    full = np.concatenate(shards, axis=0)
    return full.reshape(B, C, H, W, D)
